# revision 1
# baseline (speedup 1.0000x reference)
"""Neural ODE (64-step RK4 over a 64->256->64 ELU MLP) on 8 Trainium2 cores.

Data-parallel: batch 262144 is split into 8 shards of 32768 rows. Each core
runs the full 64-step RK4 integration on its shard entirely on-chip.

Device layout is feature-major "pair-stacked": a state tile is [128, 512]
fp32 where partitions 0-63 hold the 64 features of one 512-row batch tile
(A) and partitions 64-127 hold the features of a second batch tile (B).

Per RK4 stage f(y) = W2 @ elu(W1 y + b1) + b2:
  - mm1: 2 waves of 4 concurrent 64x64 PE-array tiles (row groups = y_A/y_B,
    col groups = two 64-wide hidden chunks) -> x = W1 y in PSUM.
  - ACT: u = exp(x + b1) (per-partition bias), one pass per wave.
  - DVE custom op: h~ = min(u,1) + relu(x + b1)  ( = elu(z) + 1 ).
  - mm2: col-tiled x2 (tile A | tile B) with pre-scaled fp16 copies of W2,
    accumulating c_i*K_i into PSUM "A" and sum_i w_i*K_i into PSUM "S".
    The elu "+1" shift is corrected via the bias row b2' = b2 - W2 @ 1.
  - State updates Y_i = Y + dt*A via fused scalar_tensor_tensor / ACT copy.
"""

import os
import sys
from contextlib import ExitStack

for _p in ("/root/.axon_site/_ro/trn_rl_repo",):
    if _p not in sys.path and os.path.isdir(_p):
        sys.path.insert(0, _p)

import numpy as np

import concourse.bass as bass
import concourse.tile as tile
from concourse import bacc, mybir
from concourse.alu_op_type import AluOpType
from concourse.bass_utils import run_bass_kernel_spmd

N_CORES = 8
BATCH = 262144
DIM = 64
HID = 256
N_STEPS = 64
SHARD = BATCH // N_CORES          # 32768
NT = 512                          # batch elems per tile (free dim)
CHUNK = 2 * NT                    # batch elems per chunk (pair-stacked)
N_CHUNKS = SHARD // CHUNK         # 32
N_PAIRS = N_CHUNKS // 2           # 16 loop iterations, 2 chunks in flight

F16 = mybir.dt.float16
F32 = mybir.dt.float32

# ---------------------------------------------------------------------------
# Custom DVE op: out = min(in0, 1) + relu(in1 + s0)
# ---------------------------------------------------------------------------

_ELUP = None


def register_elup():
    global _ELUP
    if _ELUP is not None:
        return _ELUP
    import concourse.dve_ops as D
    from concourse.dve_spec import C0, One, Spec, Src0, Src1, _has_src1, lower, minn, relu
    from concourse.dve_uop import DveOpSpec

    name = "ELUP_ANT"
    for op in D.OPS:
        if op.name == name:
            _ELUP = op
            return op
    spec = Spec(
        body=minn(Src0, One) + relu(Src1 + C0),
        reference=lambda in0, in1, s0, s1, imm2: np.minimum(
            in0.astype(np.float32), 1.0
        )
        + np.maximum(in1.astype(np.float32) + s0, 0.0),
    )
    row = 1 + len(D.OPS)
    shas = {}
    for ver in ("v3", "v4"):
        try:
            tmp = DveOpSpec(
                name=name, opcode=row, uops=lower(spec, ver=ver), rd1_en=_has_src1(spec)
            )
            shas[ver] = tmp.sha(ver)
        except Exception:
            pass
    op = D.DveOp(name, spec, subdim=False, uops_sha=shas)
    D.OPS.append(op)
    D.CUSTOM_DVE_SPECS[name] = spec
    D._SUB_OPCODE_FOR_NAME[name] = row
    _ELUP = op
    return op


# ---------------------------------------------------------------------------
# Device program
# ---------------------------------------------------------------------------


def build_ode_program(n_pairs=N_PAIRS, n_steps=N_STEPS, use_loop=True):
    """One program, run SPMD on all cores. State, weights and dt arrive
    pre-laid-out from the host."""
    elup = register_elup()
    nc = bacc.Bacc("TRN2", target_bir_lowering=False, debug=False, num_devices=1)

    ncols = n_pairs * 2 * NT
    X = nc.dram_tensor("x", [128, ncols], F32, kind="ExternalInput").ap()
    W1S = nc.dram_tensor("w1s", [128, 256], F16, kind="ExternalInput").ap()
    W2S = nc.dram_tensor("w2s", [128, 4, 256], F16, kind="ExternalInput").ap()
    BR = nc.dram_tensor("br", [1, 2, 128], F16, kind="ExternalInput").ap()
    IDT = nc.dram_tensor("idt", [128, 128], F16, kind="ExternalInput").ap()
    B1V = nc.dram_tensor("b1v", [128, 2], F32, kind="ExternalInput").ap()
    DTV = nc.dram_tensor("dtv", [128, 1], F32, kind="ExternalInput").ap()
    OUT = nc.dram_tensor("y", [128, ncols], F32, kind="ExternalOutput").ap()

    # mm2 target list per stage: (psum_name, w2_variant) ; variants:
    # 0 -> W2/2, 1 -> W2, 2 -> W2/6, 3 -> W2/3
    STAGE_TARGETS = [
        [("A", 0), ("S", 2)],  # K1: A1=(1/2)K1, S += (1/6)K1
        [("A", 0), ("S", 3)],  # K2
        [("A", 1), ("S", 3)],  # K3: A3=K3
        [("S", 2)],            # K4: S += (1/6)K4
    ]
    # bias-row variant per A_i target (BR[:,0]=b2'/2, BR[:,1]=b2')
    A_BIAS = [0, 0, 1]
    # engine for Y_i updates (i=2,3,4): "dve" = scalar_tensor_tensor,
    # "act" = identity-matmul into A + ACT scaled copy
    Y_ENGINE = ["dve", "act", "act"]

    with tile.TileContext(nc) as tc, ExitStack() as es:
        consts = es.enter_context(tc.tile_pool(name="consts", bufs=1))
        w1s = consts.tile([128, 256], F16)
        w2s = consts.tile([128, 4, 256], F16)
        br = consts.tile([1, 2, 128], F16)
        idt = consts.tile([128, 128], F16)
        b1v = consts.tile([128, 2], F32)
        dtv = consts.tile([128, 1], F32)
        ones = consts.tile([1, NT], F16)
        nc.sync.dma_start(w1s[:], W1S[:])
        nc.sync.dma_start(w2s[:], W2S[:])
        nc.sync.dma_start(br[:], BR[:])
        nc.sync.dma_start(idt[:], IDT[:])
        nc.sync.dma_start(b1v[:], B1V[:])
        nc.sync.dma_start(dtv[:], DTV[:])
        nc.vector.memset(ones[:], 1.0)

        xin_pool = es.enter_context(tc.tile_pool(name="xin", bufs=2))
        yst_pool = es.enter_context(tc.tile_pool(name="yst", bufs=4))
        yf_pool = es.enter_context(tc.tile_pool(name="yf", bufs=6))
        u_pool = es.enter_context(tc.tile_pool(name="u", bufs=4))
        h_pool = es.enter_context(tc.tile_pool(name="h", bufs=4))
        xps_pool = es.enter_context(tc.tile_pool(name="xps", bufs=2, space="PSUM"))
        aps_pool = es.enter_context(tc.tile_pool(name="aps", bufs=2, space="PSUM"))
        sps_pool = es.enter_context(tc.tile_pool(name="sps", bufs=2, space="PSUM"))

        def mm1_wave(xw, yf, w):
            """x[hidden chunkpair w] = W1_w @ y for both batch tiles.
            Two K=64, M=128 matmuls on distinct PE row groups (concurrent on
            HW); xw bank0 = batch tile A, bank1 = tile B, partitions = the
            128 hidden dims of wave w."""
            c = 128 * w
            for r in (0, 64):  # row group: 0 = tile A, 64 = tile B
                nc.tensor.matmul(
                    xw[:, 512 * (r // 64) : 512 * (r // 64) + 512],
                    w1s[r : r + 64, c : c + 128],
                    yf[r : r + 64, :],
                    start=True,
                    stop=True,
                    tile_position=(r, 0),
                    skip_group_check=True,
                )

        def mm2_wave(tgt, v, h, w, start, stop):
            """tgt[:, :] += s_v * W2_w @ h~_w  (col-tiled over batch tiles)."""
            c = 128 * w
            for d in (0, 64):  # col tile: 0 = batch tile A, 64 = tile B
                nc.tensor.matmul(
                    tgt[d : d + 64, :],
                    w2s[:, v, c + d : c + d + 64],
                    h[:, 512 * (d // 64) : 512 * (d // 64) + 512],
                    start=start,
                    stop=stop and d == 64,
                    tile_position=(0, d),
                    skip_group_check=True,
                )

        def bias_mm(tgt, v, start):
            nc.tensor.matmul(
                tgt[:, :],
                br[0:1, v, :],
                ones[0:1, :],
                start=start,
                stop=False,
                skip_group_check=True,
            )

        def stage_group(sts, i):
            """One RK4 stage for all in-flight chunks, interleaved at wave
            granularity so no engine's in-order stream head-of-line blocks
            on another chunk's dependency chain."""
            for st in sts:
                if i < 3:
                    aps_t = aps_pool.tile([128, NT], F32, tag="aps")
                    st["aps"] = aps_t
                    bias_mm(aps_t, A_BIAS[i], start=True)
            for w in (0, 1):
                for st in sts:
                    xw = xps_pool.tile([128, 2 * NT], F32, tag="xps")
                    st["xw"] = xw
                    mm1_wave(xw, st["rhs"], w)
                for st in sts:
                    u = u_pool.tile([128, 2 * NT], F16, tag="u")
                    st["u"] = u
                    nc.scalar.activation(
                        u[:],
                        st["xw"][:],
                        mybir.ActivationFunctionType.Exp,
                        bias=b1v[:, w : w + 1],
                        scale=1.0,
                    )
                for st in sts:
                    h = h_pool.tile([128, 2 * NT], F16, tag="h")
                    st["h"] = h
                    nc.vector._custom_dve(
                        elup, out=h[:], in0=st["u"][:], in1=st["xw"][:],
                        s0=b1v[:, w : w + 1],
                    )
                for st in sts:
                    for tname, v in STAGE_TARGETS[i]:
                        tgt = st["aps"] if tname == "A" else st["sps"]
                        last = w == 1 and tname == "S" and i == 3
                        mm2_wave(tgt, v, st["h"], w, start=False, stop=last)
            if i < 3:
                if Y_ENGINE[i] == "act":
                    for st in sts:
                        # A += (1/dt) * Y (identity matmul)
                        nc.tensor.matmul(
                            st["aps"][:],
                            idt[:],
                            st["yf"],
                            start=False,
                            stop=True,
                            skip_group_check=True,
                        )
                for st in sts:
                    ynext = yf_pool.tile([128, NT], F16, tag="yf")
                    if Y_ENGINE[i] == "dve":
                        nc.vector.scalar_tensor_tensor(
                            out=ynext,
                            in0=st["aps"][:],
                            scalar=dtv[:, 0:1],
                            in1=st["yst"],
                            op0=AluOpType.mult,
                            op1=AluOpType.add,
                        )
                    else:
                        nc.scalar.activation(
                            ynext,
                            st["aps"][:],
                            mybir.ActivationFunctionType.Identity,
                            bias=0.0,
                            scale=dtv[:, 0:1],
                        )
                    st["rhs"] = ynext
            else:
                for st in sts:
                    ynew = yst_pool.tile([128, NT], F32, tag="yst")
                    nc.vector.scalar_tensor_tensor(
                        out=ynew,
                        in0=st["sps"][:],
                        scalar=dtv[:, 0:1],
                        in1=st["yst"],
                        op0=AluOpType.mult,
                        op1=AluOpType.add,
                    )
                    st["yst"] = ynew

        def pair_body(col0):
            xin = xin_pool.tile([128, CHUNK], F32)
            nc.sync.dma_start(xin[:], X[:, bass.ds(col0, CHUNK)])
            sts = []
            for j in (0, 1):
                yst = xin[:, j * NT : (j + 1) * NT]
                yf = yf_pool.tile([128, NT], F16, tag="yf")
                nc.gpsimd.tensor_copy(yf, yst)
                sts.append({"yst": yst, "yf": yf, "rhs": yf, "sps": None})
            for s in range(n_steps):
                for st in sts:
                    sps_t = sps_pool.tile([128, NT], F32, tag="sps")
                    st["sps"] = sps_t
                    bias_mm(st["sps"], 1, start=True)
                for i in range(4):
                    stage_group(sts, i)
                if s < n_steps - 1:
                    for st in sts:
                        yf = yf_pool.tile([128, NT], F16, tag="yf")
                        nc.gpsimd.tensor_copy(yf, st["yst"])
                        st["yf"] = yf
                        st["rhs"] = yf
            for j in (0, 1):
                nc.sync.dma_start(OUT[:, bass.ds(col0 + j * NT, NT)], sts[j]["yst"])

        if use_loop:
            with tc.For_i(
                0,
                n_pairs * CHUNK,
                CHUNK,
                hint_engines=(
                    mybir.EngineType.PE,
                    mybir.EngineType.Activation,
                    mybir.EngineType.DVE,
                ),
            ) as col0:
                pair_body(col0)
        else:
            for p in range(n_pairs):
                pair_body(p * CHUNK)

    nc.compile()
    return nc


# ---------------------------------------------------------------------------
# Host side: prep, shard, run, gather
# ---------------------------------------------------------------------------


def _pack_state(xs):
    """[R, 64] fp32 (R batch rows) -> [128, R/2] feature-major pair-stacked."""
    r = xs.shape[0]
    t = xs.reshape(r // CHUNK, 2, NT, DIM)  # [chunks, pair, NT, 64]
    t = t.transpose(1, 3, 0, 2)             # [pair, 64, chunks, NT]
    return np.ascontiguousarray(t.reshape(2 * DIM, r // 2), dtype=np.float32)


def _unpack_state(ys, r):
    t = ys.reshape(2, DIM, r // CHUNK, NT).transpose(2, 0, 3, 1)
    return np.ascontiguousarray(t.reshape(r, DIM))


def _host_consts(t, W1, b1, W2, b2):
    dt = np.float32(np.asarray(t).reshape(-1)[0] / N_STEPS)
    W1T = W1.astype(np.float32).T  # [64, 256]
    W2T = W2.astype(np.float32).T  # [256, 64]

    w1s = np.zeros((128, 256), np.float32)
    w1s[0:64] = W1T
    w1s[64:128] = W1T

    scales = [0.5, 1.0, 1.0 / 6.0, 1.0 / 3.0]
    w2s = np.zeros((128, 4, 256), np.float32)
    for v, sc in enumerate(scales):
        for w in (0, 1):
            blk = sc * W2T[128 * w : 128 * (w + 1), :]  # [128, 64]
            w2s[:, v, 128 * w : 128 * w + 64] = blk
            w2s[:, v, 128 * w + 64 : 128 * w + 128] = blk

    b2p = b2.astype(np.float32) - W2.astype(np.float32).sum(axis=1)
    br = np.zeros((1, 2, 128), np.float32)
    br[0, 0, 0:64] = 0.5 * b2p
    br[0, 0, 64:128] = 0.5 * b2p
    br[0, 1, 0:64] = b2p
    br[0, 1, 64:128] = b2p

    idt = np.eye(128, dtype=np.float32) / dt
    b1v = b1.astype(np.float32).reshape(2, 128).T.copy()  # [:,w] = b1[128w:128w+128]
    dtv = np.full((128, 1), dt, np.float32)

    import ml_dtypes

    f16 = lambda a: a.astype(ml_dtypes.float16) if False else a.astype(np.float16)
    return {
        "w1s": f16(w1s),
        "w2s": f16(w2s),
        "br": f16(br),
        "idt": f16(idt),
        "b1v": np.ascontiguousarray(b1v, np.float32),
        "dtv": dtv,
    }


_NC_CACHE = {}


def _get_program():
    key = (N_PAIRS, N_STEPS)
    if key not in _NC_CACHE:
        _NC_CACHE[key] = build_ode_program(*key)
    return _NC_CACHE[key]


def kernel(x, t, W1, b1, W2, b2, _trace=False, _trace_kwargs=None):
    assert x.shape == (BATCH, DIM)
    nc = _get_program()
    consts = _host_consts(t, W1, b1, W2, b2)
    in_maps = []
    for c in range(N_CORES):
        shard = x[c * SHARD : (c + 1) * SHARD]
        m = {"x": _pack_state(np.asarray(shard, np.float32))}
        m.update(consts)
        in_maps.append(m)
    kw = {}
    if _trace:
        kw = {"trace": True, "trace_kwargs": _trace_kwargs or {}}
    res = run_bass_kernel_spmd(nc, in_maps, core_ids=list(range(N_CORES)), **kw)
    outs = [_unpack_state(res.results[c]["y"], SHARD) for c in range(N_CORES)]
    full = np.concatenate(outs, axis=0)
    if _trace:
        return full, res
    return full


if __name__ == "__main__":
    # quick self-check with random small data through the reference math
    rng = np.random.default_rng(0)
    x = rng.normal(size=(BATCH, DIM)).astype(np.float32)
    t = np.array([0.5], np.float32)
    s1, s2 = 1 / np.sqrt(DIM), 1 / np.sqrt(HID)
    W1 = rng.uniform(-s1, s1, (HID, DIM)).astype(np.float32)
    b1 = rng.uniform(-s1, s1, (HID,)).astype(np.float32)
    W2 = rng.uniform(-s2, s2, (DIM, HID)).astype(np.float32)
    b2 = rng.uniform(-s2, s2, (DIM,)).astype(np.float32)
    y = kernel(x=x, t=t, W1=W1, b1=b1, W2=W2, b2=b2)
    print("out", y.shape, y.dtype, np.abs(y).mean())



# revision 10
# speedup vs baseline: 1.0565x; 1.0565x over previous
"""Neural ODE (64-step RK4 over a 64->256->64 ELU MLP) on 8 Trainium2 cores.

Data-parallel: batch 262144 is split into 8 shards of 32768 rows. Each core
runs the full 64-step RK4 integration on its shard entirely on-chip.

Device layout is feature-major "pair-stacked": a state tile is [128, 512]
fp32 where partitions 0-63 hold the 64 features of one 512-row batch tile
(A) and partitions 64-127 hold the features of a second batch tile (B).

Per RK4 stage f(y) = W2 @ elu(W1 y + b1) + b2:
  - mm1: per hidden-wave w (128 hidden dims), 2 concurrent 64x64-rowgroup
    PE tiles compute z_w = W1_w y as fp16 directly into ONE PSUM bank
    [128, 1024] (halfA | halfB).
  - ACT: u_w = exp(z_w + b1_w) -> SBUF fp16.
  - DVE custom op ELUP: h~_w = min(u_w,1) + relu(z_w + b1_w)  (= elu+1).
  - mm2: col-tiled x2 with pre-scaled fp16 copies of W2, accumulating
    c_i*K_i into PSUM "A" and w_i*K_i into PSUM "S" (no bias rows; the
    elu "+1" shift and b2 are folded into per-partition bias vectors).
  - Intermediate states y_i = y + dt*c_i*(K_i + b2'): identity matmul adds
    y/dt into A, then ACT Identity with scale=dt, bias=dt*c_i*b2'.
  - Final state: DVE custom op FINUP: y' = (S + b2')*dt + y in fp32,
    plus a DVE fp16 copy for the next step's matmul operand.
"""

import os
import sys
from contextlib import ExitStack

for _p in ("/root/.axon_site/_ro/trn_rl_repo",):
    if _p not in sys.path and os.path.isdir(_p):
        sys.path.insert(0, _p)

import numpy as np

import concourse.bass as bass
import concourse.tile as tile
from concourse import bacc, mybir
from concourse.bass_utils import run_bass_kernel_spmd

N_CORES = 8
BATCH = 262144
DIM = 64
HID = 256
N_STEPS = 64
SHARD = BATCH // N_CORES          # 32768
NT = 512                          # batch elems per tile (free dim)
CHUNK = 2 * NT                    # batch elems per chunk (pair-stacked)
N_CHUNKS = SHARD // CHUNK         # 32
N_PAIRS = N_CHUNKS // 2           # 16 loop iterations, 2 chunks in flight

F16 = mybir.dt.float16
F32 = mybir.dt.float32

# ---------------------------------------------------------------------------
# Custom DVE ops
#   ELUP: out = min(in0, 1) + relu(in1 + s0)          (elu(z)+1 from u=exp)
#   FINUP: out = (in0 + s0) * s1 + in1                (final state update)
# ---------------------------------------------------------------------------

_ELUP = None
_FINUP = None


def _register_op(name, spec_body, reference):
    import concourse.dve_ops as D
    from concourse.dve_spec import Spec, _has_src1, lower
    from concourse.dve_uop import DveOpSpec

    for op in D.OPS:
        if op.name == name:
            return op
    spec = Spec(body=spec_body, reference=reference)
    row = 1 + len(D.OPS)
    shas = {}
    for ver in ("v3", "v4"):
        try:
            tmp = DveOpSpec(
                name=name, opcode=row, uops=lower(spec, ver=ver), rd1_en=_has_src1(spec)
            )
            shas[ver] = tmp.sha(ver)
        except Exception:
            pass
    op = D.DveOp(name, spec, subdim=False, uops_sha=shas)
    D.OPS.append(op)
    D.CUSTOM_DVE_SPECS[name] = spec
    D._SUB_OPCODE_FOR_NAME[name] = row
    return op


def register_ops():
    global _ELUP, _FINUP
    if _ELUP is not None:
        return _ELUP, _FINUP
    from concourse.dve_spec import C0, C1, One, Src0, Src1, minn, relu

    _ELUP = _register_op(
        "ELUP_ANT",
        minn(Src0, One) + relu(Src1 + C0),
        lambda in0, in1, s0, s1, imm2: np.minimum(in0.astype(np.float32), 1.0)
        + np.maximum(in1.astype(np.float32) + s0, 0.0),
    )
    _FINUP = _register_op(
        "FINUP_ANT",
        (Src0 + C0) * C1 + Src1,
        lambda in0, in1, s0, s1, imm2: (in0.astype(np.float32) + s0) * s1
        + in1.astype(np.float32),
    )
    return _ELUP, _FINUP


# ---------------------------------------------------------------------------
# Device program
# ---------------------------------------------------------------------------


def build_ode_program(n_pairs=N_PAIRS, n_steps=N_STEPS, use_loop=True):
    """One program, run SPMD on all cores. State, weights and dt arrive
    pre-laid-out from the host."""
    elup, finup = register_ops()
    nc = bacc.Bacc("TRN2", target_bir_lowering=False, debug=False, num_devices=1)

    ncols = n_pairs * 2 * NT
    X = nc.dram_tensor("x", [128, ncols], F32, kind="ExternalInput").ap()
    W1S = nc.dram_tensor("w1s", [128, 256], F16, kind="ExternalInput").ap()
    W2S = nc.dram_tensor("w2s", [128, 4, 256], F16, kind="ExternalInput").ap()
    IDT = nc.dram_tensor("idt", [128, 128], F16, kind="ExternalInput").ap()
    B1V = nc.dram_tensor("b1v", [128, 2], F32, kind="ExternalInput").ap()
    DTV = nc.dram_tensor("dtv", [128, 1], F32, kind="ExternalInput").ap()
    B2PV = nc.dram_tensor("b2pv", [128, 1], F32, kind="ExternalInput").ap()
    YBV = nc.dram_tensor("ybv", [128, 2], F32, kind="ExternalInput").ap()
    OUT = nc.dram_tensor("y", [128, ncols], F32, kind="ExternalOutput").ap()

    # mm2 target list per stage: (psum_name, w2_variant) ; variants:
    # 0 -> W2/2, 1 -> W2, 2 -> W2/6, 3 -> W2/3
    STAGE_TARGETS = [
        [("A", 0), ("S", 2)],  # K1: A1=(1/2)K1, S += (1/6)K1
        [("A", 0), ("S", 3)],  # K2
        [("A", 1), ("S", 3)],  # K3: A3=K3
        [("S", 2)],            # K4: S += (1/6)K4
    ]
    # ybv column per intermediate stage: dt*c_i*b2' with c = [1/2, 1/2, 1]
    A_BIAS = [0, 0, 1]

    with tile.TileContext(nc) as tc, ExitStack() as es:
        consts = es.enter_context(tc.tile_pool(name="consts", bufs=1))
        w1s = consts.tile([128, 256], F16)
        w2s = consts.tile([128, 4, 256], F16)
        idt = consts.tile([128, 128], F16)
        b1v = consts.tile([128, 2], F32)
        dtv = consts.tile([128, 1], F32)
        b2pv = consts.tile([128, 1], F32)
        ybv = consts.tile([128, 2], F32)
        nc.sync.dma_start(w1s[:], W1S[:])
        nc.sync.dma_start(w2s[:], W2S[:])
        nc.sync.dma_start(idt[:], IDT[:])
        nc.sync.dma_start(b1v[:], B1V[:])
        nc.sync.dma_start(dtv[:], DTV[:])
        nc.sync.dma_start(b2pv[:], B2PV[:])
        nc.sync.dma_start(ybv[:], YBV[:])

        xin_pool = es.enter_context(tc.tile_pool(name="xin", bufs=2))
        yst_pool = es.enter_context(tc.tile_pool(name="yst", bufs=4))
        yf_pool = es.enter_context(tc.tile_pool(name="yf", bufs=6))
        u_pool = es.enter_context(tc.tile_pool(name="u", bufs=4))
        h_pool = es.enter_context(tc.tile_pool(name="h", bufs=4))
        zps_pool = es.enter_context(tc.tile_pool(name="zps", bufs=2, space="PSUM"))
        aps_pool = es.enter_context(tc.tile_pool(name="aps", bufs=2, space="PSUM"))
        sps_pool = es.enter_context(tc.tile_pool(name="sps", bufs=2, space="PSUM"))

        def mm1_wave(zw, yf, w):
            """z_w[128 hidden, 1024] fp16 = W1_w @ y for both batch tiles.
            Two K=64, M=128 matmuls on distinct PE row groups (concurrent on
            HW); columns 0:512 = batch tile A, 512:1024 = tile B."""
            c = 128 * w
            for r in (0, 64):
                nc.tensor.matmul(
                    zw[:, 512 * (r // 64) : 512 * (r // 64) + 512],
                    w1s[r : r + 64, c : c + 128],
                    yf[r : r + 64, :],
                    start=True,
                    stop=True,
                    tile_position=(r, 0),
                    skip_group_check=True,
                )

        def mm2_wave(tgt, v, h, w, start, stop):
            """tgt[:, :] += s_v * W2_w @ h~_w  (col-tiled over batch tiles)."""
            c = 128 * w
            for d in (0, 64):
                nc.tensor.matmul(
                    tgt[d : d + 64, :],
                    w2s[:, v, c + d : c + d + 64],
                    h[:, 512 * (d // 64) : 512 * (d // 64) + 512],
                    start=start,
                    stop=stop,
                    tile_position=(0, d),
                    skip_group_check=True,
                )

        def stage_group(sts, i):
            """One RK4 stage for both in-flight chunks, interleaved at wave
            granularity."""
            for st in sts:
                st["zw"] = [None, None]
                st["u"] = [None, None]
                st["h"] = [None, None]
                if i < 3:
                    aps_t = aps_pool.tile([128, NT], F32, tag="aps")
                    st["aps"] = aps_t
            for w in (0, 1):
                for st in sts:
                    zw = zps_pool.tile([128, 2 * NT], F32, tag="zps")
                    st["zw"][w] = zw
                    mm1_wave(zw, st["rhs"], w)
                for st in sts:
                    u = u_pool.tile([128, 2 * NT], F16, tag="u")
                    st["u"][w] = u
                    nc.scalar.activation(
                        u[:],
                        st["zw"][w][:],
                        mybir.ActivationFunctionType.Exp,
                        bias=b1v[:, w : w + 1],
                        scale=1.0,
                    )
                for st in sts:
                    h = h_pool.tile([128, 2 * NT], F16, tag="h")
                    st["h"][w] = h
                    nc.vector._custom_dve(
                        elup, out=h[:], in0=st["u"][w][:], in1=st["zw"][w][:],
                        s0=b1v[:, w : w + 1],
                    )
                for st in sts:
                    for tname, v in STAGE_TARGETS[i]:
                        tgt = st["aps"] if tname == "A" else st["sps"]
                        first = w == 0 and (tname == "A" or i == 0)
                        last = w == 1 and tname == "S" and i == 3
                        mm2_wave(tgt, v, st["h"][w], w, start=first, stop=last)
            if i < 3:
                for st in sts:
                    # A += (1/dt) * y  (identity matmul on base state)
                    nc.tensor.matmul(
                        st["aps"][:],
                        idt[:],
                        st["yf"],
                        start=False,
                        stop=True,
                        skip_group_check=True,
                    )
                for st in sts:
                    ynext = yf_pool.tile([128, NT], F16, tag="yf")
                    nc.scalar.activation(
                        ynext,
                        st["aps"][:],
                        mybir.ActivationFunctionType.Identity,
                        bias=ybv[:, A_BIAS[i] : A_BIAS[i] + 1],
                        scale=dtv[:, 0:1],
                    )
                    st["rhs"] = ynext
            else:
                for st in sts:
                    ynew = yst_pool.tile([128, NT], F32, tag="yst")
                    nc.vector._custom_dve(
                        finup,
                        out=ynew,
                        in0=st["sps"][:],
                        in1=st["yst"],
                        s0=b2pv[:, 0:1],
                        s1=dtv[:, 0:1],
                    )
                    st["yst"] = ynew

        def pair_body(col0):
            xin = xin_pool.tile([128, CHUNK], F32)
            nc.sync.dma_start(xin[:], X[:, bass.ds(col0, CHUNK)])
            sts = []
            for j in (0, 1):
                yst = xin[:, j * NT : (j + 1) * NT]
                yf = yf_pool.tile([128, NT], F16, tag="yf")
                nc.vector.tensor_copy(yf, yst)
                sts.append({"yst": yst, "yf": yf, "rhs": yf, "sps": None})
            for s in range(n_steps):
                for st in sts:
                    sps_t = sps_pool.tile([128, NT], F32, tag="sps")
                    st["sps"] = sps_t
                for i in range(4):
                    stage_group(sts, i)
                if s < n_steps - 1:
                    for st in sts:
                        yf = yf_pool.tile([128, NT], F16, tag="yf")
                        nc.vector.tensor_copy(yf, st["yst"])
                        st["yf"] = yf
                        st["rhs"] = yf
            for j in (0, 1):
                nc.sync.dma_start(OUT[:, bass.ds(col0 + j * NT, NT)], sts[j]["yst"])

        if use_loop:
            with tc.For_i(
                0,
                n_pairs * CHUNK,
                CHUNK,
                hint_engines=(
                    mybir.EngineType.PE,
                    mybir.EngineType.Activation,
                    mybir.EngineType.DVE,
                ),
            ) as col0:
                pair_body(col0)
        else:
            for p in range(n_pairs):
                pair_body(p * CHUNK)

    nc.compile()
    return nc


# ---------------------------------------------------------------------------
# Host side: prep, shard, run, gather
# ---------------------------------------------------------------------------


def _pack_state(xs):
    """[R, 64] fp32 (R batch rows) -> [128, R/2] feature-major pair-stacked."""
    r = xs.shape[0]
    t = xs.reshape(r // CHUNK, 2, NT, DIM)  # [chunks, pair, NT, 64]
    t = t.transpose(1, 3, 0, 2)             # [pair, 64, chunks, NT]
    return np.ascontiguousarray(t.reshape(2 * DIM, r // 2), dtype=np.float32)


def _unpack_state(ys, r):
    t = ys.reshape(2, DIM, r // CHUNK, NT).transpose(2, 0, 3, 1)
    return np.ascontiguousarray(t.reshape(r, DIM))


def _host_consts(t, W1, b1, W2, b2):
    dt = np.float32(np.asarray(t).reshape(-1)[0] / N_STEPS)
    W1T = W1.astype(np.float32).T  # [64, 256]
    W2T = W2.astype(np.float32).T  # [256, 64]

    w1s = np.zeros((128, 256), np.float32)
    w1s[0:64] = W1T
    w1s[64:128] = W1T

    scales = [0.5, 1.0, 1.0 / 6.0, 1.0 / 3.0]
    w2s = np.zeros((128, 4, 256), np.float32)
    for v, sc in enumerate(scales):
        for w in (0, 1):
            blk = sc * W2T[128 * w : 128 * (w + 1), :]  # [128, 64]
            w2s[:, v, 128 * w : 128 * w + 64] = blk
            w2s[:, v, 128 * w + 64 : 128 * w + 128] = blk

    b2p = b2.astype(np.float32) - W2.astype(np.float32).sum(axis=1)
    b2ps = np.concatenate([b2p, b2p])  # [128] pair-stacked

    idt = np.eye(128, dtype=np.float32) / dt
    b1v = b1.astype(np.float32).reshape(2, 128).T.copy()  # [:,w] = b1[128w:128w+128]
    dtv = np.full((128, 1), dt, np.float32)
    b2pv = b2ps.reshape(128, 1).astype(np.float32)
    ybv = np.stack([dt * 0.5 * b2ps, dt * b2ps], axis=1).astype(np.float32)

    f16 = lambda a: a.astype(np.float16)
    return {
        "w1s": f16(w1s),
        "w2s": f16(w2s),
        "idt": f16(idt),
        "b1v": np.ascontiguousarray(b1v, np.float32),
        "dtv": dtv,
        "b2pv": np.ascontiguousarray(b2pv, np.float32),
        "ybv": np.ascontiguousarray(ybv, np.float32),
    }


_NC_CACHE = {}


def _get_program():
    key = (N_PAIRS, N_STEPS)
    if key not in _NC_CACHE:
        _NC_CACHE[key] = build_ode_program(*key)
    return _NC_CACHE[key]


def kernel(x, t, W1, b1, W2, b2, _trace=False, _trace_kwargs=None):
    assert x.shape == (BATCH, DIM)
    nc = _get_program()
    consts = _host_consts(t, W1, b1, W2, b2)
    in_maps = []
    for c in range(N_CORES):
        shard = x[c * SHARD : (c + 1) * SHARD]
        m = {"x": _pack_state(np.asarray(shard, np.float32))}
        m.update(consts)
        in_maps.append(m)
    kw = {}
    if _trace:
        kw = {"trace": True, "trace_kwargs": _trace_kwargs or {}}
    res = run_bass_kernel_spmd(nc, in_maps, core_ids=list(range(N_CORES)), **kw)
    outs = [_unpack_state(res.results[c]["y"], SHARD) for c in range(N_CORES)]
    full = np.concatenate(outs, axis=0)
    if _trace:
        return full, res
    return full


if __name__ == "__main__":
    rng = np.random.default_rng(0)
    x = rng.normal(size=(BATCH, DIM)).astype(np.float32)
    t = np.array([0.5], np.float32)
    s1, s2 = 1 / np.sqrt(DIM), 1 / np.sqrt(HID)
    W1 = rng.uniform(-s1, s1, (HID, DIM)).astype(np.float32)
    b1 = rng.uniform(-s1, s1, (HID,)).astype(np.float32)
    W2 = rng.uniform(-s2, s2, (DIM, HID)).astype(np.float32)
    b2 = rng.uniform(-s2, s2, (DIM,)).astype(np.float32)
    y = kernel(x=x, t=t, W1=W1, b1=b1, W2=W2, b2=b2)
    print("out", y.shape, y.dtype, np.abs(y).mean())


# revision 15
# speedup vs baseline: 1.5247x; 1.4431x over previous
"""Neural ODE (64-step RK4 over a 64->256->64 ELU MLP) on 8 Trainium2 cores.

Data-parallel: batch 262144 is split into 8 shards of 32768 rows. Each core
runs the full 64-step RK4 integration on its shard entirely on-chip.

Device layout is feature-major "pair-stacked": a state tile is [128, 512]
fp32 where partitions 0-63 hold the 64 features of one 512-row batch tile
(A) and partitions 64-127 hold the features of a second batch tile (B).

The ELU is evaluated in a SINGLE ScalarE pass using a patched activation
table: the `exp` entry of the `exp_and_others` PWP set is rewritten so that
its positive-x buckets compute the exact linear 1+x while the negative-x
buckets keep the stock exp spline. The resulting function is
    elup1(x) = exp(x)      for x <= 0
             = 1 + x       for x >  0        ( = elu(x) + 1 )
with zero/inf/nan behavior matching elu+1 as well. h~ = elup1(z + b1) comes
straight out of ACT as fp16; the "+1" shift is corrected through the bias
b2' = b2 - W2 @ 1 folded into the DVE state updates.

Per RK4 stage f(y) = W2 @ elu(W1 y + b1) + b2:
  - mm1: 2 waves of 2 concurrent 64-rowgroup PE tiles -> z = W1 y in PSUM.
  - ACT: h~ = elup1(z + b1) -> SBUF fp16 (one pass, no DVE combine).
  - mm2: col-tiled x2 with pre-scaled fp16 copies of W2, accumulating
    c_i*K_i into PSUM "A" and w_i*K_i into PSUM "S".
  - State updates on DVE via custom FINUP op: out = (in0 + s0)*s1 + in1,
    i.e. y_i = (A + c_i b2')*dt + y, all biases via per-partition scalars.
"""

import os
import shutil
import sys
import tempfile
from contextlib import ExitStack

for _p in ("/root/.axon_site/_ro/trn_rl_repo",):
    if _p not in sys.path and os.path.isdir(_p):
        sys.path.insert(0, _p)

import numpy as np

import concourse.bass as bass
import concourse.tile as tile
from concourse import bacc, mybir
from concourse.bass_utils import run_bass_kernel_spmd

N_CORES = 8
BATCH = 262144
DIM = 64
HID = 256
N_STEPS = 64
SHARD = BATCH // N_CORES          # 32768
NT = 512                          # batch elems per tile (free dim)
CHUNK = 2 * NT                    # batch elems per chunk (pair-stacked)
N_CHUNKS = SHARD // CHUNK         # 32 chunks of [128, 512]
GROUP = 3                         # chunks in flight per loop iteration
N_GROUPS = 10                     # For_i iterations; tail of 2 chunks after

F16 = mybir.dt.float16
F32 = mybir.dt.float32

# ---------------------------------------------------------------------------
# Patched activation tables: exp -> elup1 (= elu + 1)
# ---------------------------------------------------------------------------

_ACT_ROOT = None


def forge_act_root():
    """Build a private copy of the PWP activation tables in which the
    positive-x buckets of `exp` (exp_and_others set) evaluate the exact
    linear 1+x. Returns the path of the patched act_info.json."""
    global _ACT_ROOT
    if _ACT_ROOT is not None:
        return _ACT_ROOT
    import json

    from neuronxcc.driver.Job import Job
    from neuronxcc.driver.jobs.support.FindActInfo import findActInfoFile

    src = os.path.dirname(findActInfoFile(Job.getPackageDir(), "gen3"))
    dst = os.path.join(tempfile.mkdtemp(prefix="elup1_act_"), "pwp_bin_trainium")
    shutil.copytree(src, dst)

    prof = json.load(open(os.path.join(dst, "exp_and_others.json")))
    starts = prof["func_to_bkt_start_idx"]
    s = starts["exp"]
    e = min(v for v in starts.values() if v > s)  # next function's start

    path = os.path.join(dst, "exp_and_others_bkt.bin")
    a = np.frombuffer(open(path, "rb").read(), dtype=np.float32).reshape(-1, 8).copy()
    blk = a[s:e]
    pos = blk[:, 4] > 0
    blk[pos, 0] = 1.0 + blk[pos, 4]   # c0 = 1 + x0
    blk[pos, 1] = 1.0                 # c1 = 1
    blk[pos, 2] = 0.0
    blk[pos, 3] = 0.0
    sat = np.isinf(blk[:, 0])         # +overflow saturation bucket -> 1 + x
    blk[sat, 0] = 1.0
    blk[sat, 1] = 1.0
    blk[sat, 2] = 0.0
    blk[sat, 3] = 0.0
    a[s:e] = blk
    with open(path, "wb") as f:
        f.write(a.tobytes())

    _ACT_ROOT = os.path.join(dst, "act_info.json")
    return _ACT_ROOT


# ---------------------------------------------------------------------------
# Custom DVE op: FINUP: out = (in0 + s0) * s1 + in1
# ---------------------------------------------------------------------------

_FINUP = None


def register_finup():
    global _FINUP
    if _FINUP is not None:
        return _FINUP
    import concourse.dve_ops as D
    from concourse.dve_spec import C0, C1, Spec, Src0, Src1, _has_src1, lower
    from concourse.dve_uop import DveOpSpec

    name = "FINUP_ANT"
    for op in D.OPS:
        if op.name == name:
            _FINUP = op
            return op
    spec = Spec(
        body=(Src0 + C0) * C1 + Src1,
        reference=lambda in0, in1, s0, s1, imm2: (in0.astype(np.float32) + s0) * s1
        + in1.astype(np.float32),
    )
    row = 1 + len(D.OPS)
    shas = {}
    for ver in ("v3", "v4"):
        try:
            tmp = DveOpSpec(
                name=name, opcode=row, uops=lower(spec, ver=ver), rd1_en=_has_src1(spec)
            )
            shas[ver] = tmp.sha(ver)
        except Exception:
            pass
    op = D.DveOp(name, spec, subdim=False, uops_sha=shas)
    D.OPS.append(op)
    D.CUSTOM_DVE_SPECS[name] = spec
    D._SUB_OPCODE_FOR_NAME[name] = row
    _FINUP = op
    return op


# ---------------------------------------------------------------------------
# Device program
# ---------------------------------------------------------------------------


def build_ode_program(n_steps=N_STEPS, use_loop=True):
    """One program, run SPMD on all cores. State, weights and dt arrive
    pre-laid-out from the host."""
    finup = register_finup()
    nc = bacc.Bacc("TRN2", target_bir_lowering=False, debug=False, num_devices=1)

    ncols = N_CHUNKS * NT
    X = nc.dram_tensor("x", [128, ncols], F32, kind="ExternalInput").ap()
    W1S = nc.dram_tensor("w1s", [128, 256], F16, kind="ExternalInput").ap()
    W2S = nc.dram_tensor("w2s", [128, 4, 256], F16, kind="ExternalInput").ap()
    B1V = nc.dram_tensor("b1v", [128, 2], F32, kind="ExternalInput").ap()
    DTV = nc.dram_tensor("dtv", [128, 1], F32, kind="ExternalInput").ap()
    CBV = nc.dram_tensor("cbv", [128, 2], F32, kind="ExternalInput").ap()
    OUT = nc.dram_tensor("y", [128, ncols], F32, kind="ExternalOutput").ap()

    # mm2 target list per stage: (psum_name, w2_variant) ; variants:
    # 0 -> W2/2, 1 -> W2, 2 -> W2/6, 3 -> W2/3
    STAGE_TARGETS = [
        [("A", 0), ("S", 2)],  # K1: A1=(1/2)K1, S += (1/6)K1
        [("A", 0), ("S", 3)],  # K2
        [("A", 1), ("S", 3)],  # K3: A3=K3
        [("S", 2)],            # K4: S += (1/6)K4
    ]
    # cbv column per intermediate stage: c_i*b2' with c = [1/2, 1/2, 1]
    A_BIAS = [0, 0, 1]

    with tile.TileContext(nc) as tc, ExitStack() as es:
        consts = es.enter_context(tc.tile_pool(name="consts", bufs=1))
        w1s = consts.tile([128, 256], F16)
        w2s = consts.tile([128, 4, 256], F16)
        b1v = consts.tile([128, 2], F32)
        dtv = consts.tile([128, 1], F32)
        cbv = consts.tile([128, 2], F32)
        nc.sync.dma_start(w1s[:], W1S[:])
        nc.sync.dma_start(w2s[:], W2S[:])
        nc.sync.dma_start(b1v[:], B1V[:])
        nc.sync.dma_start(dtv[:], DTV[:])
        nc.sync.dma_start(cbv[:], CBV[:])

        xin_pool = es.enter_context(tc.tile_pool(name="xin", bufs=2))
        yst_pool = es.enter_context(tc.tile_pool(name="yst", bufs=7))
        yf_pool = es.enter_context(tc.tile_pool(name="yf", bufs=10))
        h_pool = es.enter_context(tc.tile_pool(name="h", bufs=20))
        zps_pool = es.enter_context(tc.tile_pool(name="zps", bufs=3, space="PSUM"))
        aps_pool = es.enter_context(tc.tile_pool(name="aps", bufs=2, space="PSUM"))
        sps_pool = es.enter_context(tc.tile_pool(name="sps", bufs=3, space="PSUM"))

        TOKS = [(w, d) for w in (0, 1) for d in (0, 64)]

        def mm1_tok(z, yf, w, d):
            """z[128 hidden of wave w, 512] = W1_w @ y for batch half d."""
            c = 128 * w
            nc.tensor.matmul(
                z[:, :],
                w1s[d : d + 64, c : c + 128],
                yf[d : d + 64, :],
                start=True,
                stop=True,
                tile_position=(d, 0),
                skip_group_check=True,
            )

        def mm2_tok(tgt, v, h, w, d, start, stop):
            """tgt[half d] += s_v * W2_w @ h~(w,d)."""
            c = 128 * w
            nc.tensor.matmul(
                tgt[d : d + 64, :],
                w2s[:, v, c + d : c + d + 64],
                h[:, :],
                start=start,
                stop=stop,
                tile_position=(0, d),
                skip_group_check=True,
            )

        def stage_group(sts, i, deferred):
            """One RK4 stage for the in-flight chunks at (wave, half) token
            granularity. The S-target matmuls of stage i-1 (in `deferred`)
            are emitted after this stage's mm1s so the PE prioritizes z
            production; they only matter at the end of the step."""
            for st in sts:
                st["z"] = {}
                st["h"] = {}
                if i < 3:
                    aps_t = aps_pool.tile([128, NT], F32, tag="aps")
                    st["aps"] = aps_t
            for st in sts:
                for w, d in TOKS:
                    z = zps_pool.tile([128, NT], F32, tag="zps")
                    st["z"][(w, d)] = z
                    mm1_tok(z, st["rhs"], w, d)
            for st in sts:
                for w, d in TOKS:
                    # h~ = elup1(z + b1) in one ACT pass (patched exp table)
                    h = h_pool.tile([128, NT], F16, tag="h")
                    st["h"][(w, d)] = h
                    nc.scalar.activation(
                        h[:],
                        st["z"][(w, d)][:],
                        mybir.ActivationFunctionType.Exp,
                        bias=b1v[:, w : w + 1],
                        scale=1.0,
                    )
            for emit in deferred:
                emit()
            deferred.clear()
            sv = [v for tname, v in STAGE_TARGETS[i] if tname == "S"][0]
            for st in sts:
                hs = st["h"]
                if i < 3:
                    av = [v for tname, v in STAGE_TARGETS[i] if tname == "A"][0]
                    for w, d in TOKS:
                        mm2_tok(st["aps"], av, hs[(w, d)], w, d, start=w == 0, stop=w == 1)
                def emit_s(st=st, hs=hs, i=i, sv=sv):
                    for w, d in TOKS:
                        mm2_tok(
                            st["sps"], sv, hs[(w, d)], w, d,
                            start=(i == 0 and w == 0),
                            stop=(i == 3 and w == 1),
                        )
                if i == 3:
                    emit_s()
                else:
                    deferred.append(emit_s)
            if i < 3:
                for st in sts:
                    # y_i = (A + c_i b2')*dt + y   (fp16, feeds next mm1)
                    ynext = yf_pool.tile([128, NT], F16, tag="yf")
                    nc.vector._custom_dve(
                        finup,
                        out=ynext,
                        in0=st["aps"][:],
                        in1=st["yf"],
                        s0=cbv[:, A_BIAS[i] : A_BIAS[i] + 1],
                        s1=dtv[:, 0:1],
                    )
                    st["rhs"] = ynext
            else:
                for st in sts:
                    # next step's fp16 base first (critical path into mm1) ...
                    ynf = yf_pool.tile([128, NT], F16, tag="yf")
                    nc.vector._custom_dve(
                        finup,
                        out=ynf,
                        in0=st["sps"][:],
                        in1=st["yst"],
                        s0=cbv[:, 1:2],
                        s1=dtv[:, 0:1],
                    )
                    st["next_yf"] = ynf
                for st in sts:
                    # ... then the fp32 master state off the critical path
                    ynew = yst_pool.tile([128, NT], F32, tag="yst")
                    nc.vector._custom_dve(
                        finup,
                        out=ynew,
                        in0=st["sps"][:],
                        in1=st["yst"],
                        s0=cbv[:, 1:2],
                        s1=dtv[:, 0:1],
                    )
                    st["yst"] = ynew

        def group_body(col0, n_in_group):
            xin = xin_pool.tile([128, GROUP * NT], F32, tag="xin")
            nc.sync.dma_start(
                xin[:, 0 : n_in_group * NT], X[:, bass.ds(col0, n_in_group * NT)]
            )
            sts = []
            for j in range(n_in_group):
                yst = xin[:, j * NT : (j + 1) * NT]
                yf = yf_pool.tile([128, NT], F16, tag="yf")
                nc.vector.tensor_copy(yf, yst)
                sts.append({"yst": yst, "yf": yf, "rhs": yf, "sps": None})
            deferred = []
            for s in range(n_steps):
                for st in sts:
                    sps_t = sps_pool.tile([128, NT], F32, tag="sps")
                    st["sps"] = sps_t
                for i in range(4):
                    stage_group(sts, i, deferred)
                if s < n_steps - 1:
                    for st in sts:
                        st["yf"] = st["next_yf"]
                        st["rhs"] = st["next_yf"]
            for j in range(n_in_group):
                nc.sync.dma_start(OUT[:, bass.ds(col0 + j * NT, NT)], sts[j]["yst"])

        if use_loop:
            with tc.For_i(
                0,
                N_GROUPS * GROUP * NT,
                GROUP * NT,
                hint_engines=(
                    mybir.EngineType.PE,
                    mybir.EngineType.Activation,
                    mybir.EngineType.DVE,
                ),
            ) as col0:
                group_body(col0, GROUP)
        else:
            for g in range(N_GROUPS):
                group_body(g * GROUP * NT, GROUP)
        tail = N_CHUNKS - N_GROUPS * GROUP
        if tail:
            group_body(N_GROUPS * GROUP * NT, tail)

    nc.compile()
    return nc


# ---------------------------------------------------------------------------
# Host side: prep, shard, run, gather
# ---------------------------------------------------------------------------


def _pack_state(xs):
    """[R, 64] fp32 (R batch rows) -> [128, R/2] feature-major pair-stacked."""
    r = xs.shape[0]
    t = xs.reshape(r // CHUNK, 2, NT, DIM)  # [chunks, pair, NT, 64]
    t = t.transpose(1, 3, 0, 2)             # [pair, 64, chunks, NT]
    return np.ascontiguousarray(t.reshape(2 * DIM, r // 2), dtype=np.float32)


def _unpack_state(ys, r):
    t = ys.reshape(2, DIM, r // CHUNK, NT).transpose(2, 0, 3, 1)
    return np.ascontiguousarray(t.reshape(r, DIM))


def _host_consts(t, W1, b1, W2, b2):
    dt = np.float32(np.asarray(t).reshape(-1)[0] / N_STEPS)
    W1T = W1.astype(np.float32).T  # [64, 256]
    W2T = W2.astype(np.float32).T  # [256, 64]

    w1s = np.zeros((128, 256), np.float32)
    w1s[0:64] = W1T
    w1s[64:128] = W1T

    scales = [0.5, 1.0, 1.0 / 6.0, 1.0 / 3.0]
    w2s = np.zeros((128, 4, 256), np.float32)
    for v, sc in enumerate(scales):
        for w in (0, 1):
            blk = sc * W2T[128 * w : 128 * (w + 1), :]  # [128, 64]
            w2s[:, v, 128 * w : 128 * w + 64] = blk
            w2s[:, v, 128 * w + 64 : 128 * w + 128] = blk

    b2p = b2.astype(np.float32) - W2.astype(np.float32).sum(axis=1)
    b2ps = np.concatenate([b2p, b2p])  # [128] pair-stacked

    b1v = b1.astype(np.float32).reshape(2, 128).T.copy()  # [:,w] = b1[128w:128w+128]
    dtv = np.full((128, 1), dt, np.float32)
    cbv = np.stack([0.5 * b2ps, b2ps], axis=1).astype(np.float32)

    f16 = lambda a: a.astype(np.float16)
    return {
        "w1s": f16(w1s),
        "w2s": f16(w2s),
        "b1v": np.ascontiguousarray(b1v, np.float32),
        "dtv": dtv,
        "cbv": np.ascontiguousarray(cbv, np.float32),
    }


_NC_CACHE = {}


def _get_program():
    key = (N_GROUPS, GROUP, N_STEPS)
    if key not in _NC_CACHE:
        _NC_CACHE[key] = build_ode_program()
    return _NC_CACHE[key]


def kernel(x, t, W1, b1, W2, b2, _trace=False, _trace_kwargs=None):
    assert x.shape == (BATCH, DIM)
    nc = _get_program()
    consts = _host_consts(t, W1, b1, W2, b2)
    in_maps = []
    for c in range(N_CORES):
        shard = x[c * SHARD : (c + 1) * SHARD]
        m = {"x": _pack_state(np.asarray(shard, np.float32))}
        m.update(consts)
        in_maps.append(m)
    kw = {}
    if _trace:
        kw = {"trace": True, "trace_kwargs": _trace_kwargs or {}}
    # The patched table must be visible to the neuronx-cc invocation that the
    # first execution triggers; restore the env afterwards so no other jax
    # compile in this process picks it up.
    prev = os.environ.get("BASS_ACT_ROOT_JSON_PATH")
    os.environ["BASS_ACT_ROOT_JSON_PATH"] = forge_act_root()
    try:
        res = run_bass_kernel_spmd(nc, in_maps, core_ids=list(range(N_CORES)), **kw)
    finally:
        if prev is None:
            os.environ.pop("BASS_ACT_ROOT_JSON_PATH", None)
        else:
            os.environ["BASS_ACT_ROOT_JSON_PATH"] = prev
    outs = [_unpack_state(res.results[c]["y"], SHARD) for c in range(N_CORES)]
    full = np.concatenate(outs, axis=0)
    if _trace:
        return full, res
    return full


if __name__ == "__main__":
    rng = np.random.default_rng(0)
    x = rng.normal(size=(BATCH, DIM)).astype(np.float32)
    t = np.array([0.5], np.float32)
    s1, s2 = 1 / np.sqrt(DIM), 1 / np.sqrt(HID)
    W1 = rng.uniform(-s1, s1, (HID, DIM)).astype(np.float32)
    b1 = rng.uniform(-s1, s1, (HID,)).astype(np.float32)
    W2 = rng.uniform(-s2, s2, (DIM, HID)).astype(np.float32)
    b2 = rng.uniform(-s2, s2, (DIM,)).astype(np.float32)
    y = kernel(x=x, t=t, W1=W1, b1=b1, W2=W2, b2=b2)
    print("out", y.shape, y.dtype, np.abs(y).mean())


# revision 16
# speedup vs baseline: 1.8477x; 1.2119x over previous
"""Neural ODE (64-step RK4 over a 64->256->64 ELU MLP) on 8 Trainium2 cores.

Data-parallel: batch 262144 is split into 8 shards of 32768 rows. Each core
runs the full 64-step RK4 integration on its shard entirely on-chip.

Device layout is feature-major "pair-stacked": a state tile is [128, 512]
fp32 where partitions 0-63 hold the 64 features of one 512-row batch tile
(A) and partitions 64-127 hold the features of a second batch tile (B).

The ELU is evaluated in a SINGLE ScalarE pass using a patched activation
table: the `exp` entry of the `exp_and_others` PWP set is rewritten so that
its positive-x buckets compute the exact linear 1+x while the negative-x
buckets keep the stock exp spline. The resulting function is
    elup1(x) = exp(x)      for x <= 0
             = 1 + x       for x >  0        ( = elu(x) + 1 )
with zero/inf/nan behavior matching elu+1 as well. h~ = elup1(z + b1) comes
straight out of ACT as fp16; the "+1" shift is corrected through the bias
b2' = b2 - W2 @ 1 folded into the DVE state updates.

Per RK4 stage f(y) = W2 @ elu(W1 y + b1) + b2:
  - mm1: 2 waves of 2 concurrent 64-rowgroup PE tiles -> z = W1 y in PSUM.
  - ACT: h~ = elup1(z + b1) -> SBUF fp16 (one pass, no DVE combine).
  - mm2: col-tiled x2 with pre-scaled fp16 copies of W2, accumulating
    c_i*K_i into PSUM "A" and w_i*K_i into PSUM "S".
  - State updates on DVE via custom FINUP op: out = (in0 + s0)*s1 + in1,
    i.e. y_i = (A + c_i b2')*dt + y, all biases via per-partition scalars.
"""

import os
import shutil
import sys
import tempfile
from contextlib import ExitStack

for _p in ("/root/.axon_site/_ro/trn_rl_repo",):
    if _p not in sys.path and os.path.isdir(_p):
        sys.path.insert(0, _p)

import numpy as np

import concourse.bass as bass
import concourse.tile as tile
from concourse import bacc, mybir
from concourse.bass_utils import run_bass_kernel_spmd

N_CORES = 8
BATCH = 262144
DIM = 64
HID = 256
N_STEPS = 64
SHARD = BATCH // N_CORES          # 32768
NT = 512                          # batch elems per tile (free dim)
CHUNK = 2 * NT                    # batch elems per chunk (pair-stacked)
N_CHUNKS = SHARD // CHUNK         # 32 chunks of [128, 512]
GROUP = 3                         # chunks in flight per loop iteration
N_GROUPS = 10                     # For_i iterations; tail of 2 chunks after

F16 = mybir.dt.float16
F32 = mybir.dt.float32

# ---------------------------------------------------------------------------
# Patched activation tables: exp -> elup1 (= elu + 1)
# ---------------------------------------------------------------------------

_ACT_ROOT = None


def forge_act_root():
    """Build a private copy of the PWP activation tables in which the
    positive-x buckets of `exp` (exp_and_others set) evaluate the exact
    linear 1+x. Returns the path of the patched act_info.json."""
    global _ACT_ROOT
    if _ACT_ROOT is not None:
        return _ACT_ROOT
    import json

    from neuronxcc.driver.Job import Job
    from neuronxcc.driver.jobs.support.FindActInfo import findActInfoFile

    src = os.path.dirname(findActInfoFile(Job.getPackageDir(), "gen3"))
    dst = os.path.join(tempfile.mkdtemp(prefix="elup1_act_"), "pwp_bin_trainium")
    shutil.copytree(src, dst)

    prof = json.load(open(os.path.join(dst, "exp_and_others.json")))
    starts = prof["func_to_bkt_start_idx"]
    s = starts["exp"]
    e = min(v for v in starts.values() if v > s)  # next function's start

    path = os.path.join(dst, "exp_and_others_bkt.bin")
    a = np.frombuffer(open(path, "rb").read(), dtype=np.float32).reshape(-1, 8).copy()
    blk = a[s:e]
    pos = blk[:, 4] > 0
    blk[pos, 0] = 1.0 + blk[pos, 4]   # c0 = 1 + x0
    blk[pos, 1] = 1.0                 # c1 = 1
    blk[pos, 2] = 0.0
    blk[pos, 3] = 0.0
    sat = np.isinf(blk[:, 0])         # +overflow saturation bucket -> 1 + x
    blk[sat, 0] = 1.0
    blk[sat, 1] = 1.0
    blk[sat, 2] = 0.0
    blk[sat, 3] = 0.0
    a[s:e] = blk
    with open(path, "wb") as f:
        f.write(a.tobytes())

    _ACT_ROOT = os.path.join(dst, "act_info.json")
    return _ACT_ROOT


# ---------------------------------------------------------------------------
# Custom DVE op: FINUP: out = (in0 + s0) * s1 + in1
# ---------------------------------------------------------------------------

_FINUP = None


def register_finup():
    global _FINUP
    if _FINUP is not None:
        return _FINUP
    import concourse.dve_ops as D
    from concourse.dve_spec import C0, C1, Spec, Src0, Src1, _has_src1, lower
    from concourse.dve_uop import DveOpSpec

    name = "FINUP_ANT"
    for op in D.OPS:
        if op.name == name:
            _FINUP = op
            return op
    spec = Spec(
        body=(Src0 + C0) * C1 + Src1,
        reference=lambda in0, in1, s0, s1, imm2: (in0.astype(np.float32) + s0) * s1
        + in1.astype(np.float32),
    )
    row = 1 + len(D.OPS)
    shas = {}
    for ver in ("v3", "v4"):
        try:
            tmp = DveOpSpec(
                name=name, opcode=row, uops=lower(spec, ver=ver), rd1_en=_has_src1(spec)
            )
            shas[ver] = tmp.sha(ver)
        except Exception:
            pass
    op = D.DveOp(name, spec, subdim=False, uops_sha=shas)
    D.OPS.append(op)
    D.CUSTOM_DVE_SPECS[name] = spec
    D._SUB_OPCODE_FOR_NAME[name] = row
    _FINUP = op
    return op


# ---------------------------------------------------------------------------
# Device program
# ---------------------------------------------------------------------------


def build_ode_program(n_steps=N_STEPS, use_loop=True):
    """One program, run SPMD on all cores. State, weights and dt arrive
    pre-laid-out from the host."""
    finup = register_finup()
    nc = bacc.Bacc("TRN2", target_bir_lowering=False, debug=False, num_devices=1)

    ncols = N_CHUNKS * NT
    X = nc.dram_tensor("x", [128, ncols], F32, kind="ExternalInput").ap()
    W1S = nc.dram_tensor("w1s", [128, 256], F16, kind="ExternalInput").ap()
    W2S = nc.dram_tensor("w2s", [128, 4, 256], F16, kind="ExternalInput").ap()
    B1V = nc.dram_tensor("b1v", [128, 2], F32, kind="ExternalInput").ap()
    DTV = nc.dram_tensor("dtv", [128, 1], F32, kind="ExternalInput").ap()
    CBV = nc.dram_tensor("cbv", [128, 2], F32, kind="ExternalInput").ap()
    OUT = nc.dram_tensor("y", [128, ncols], F32, kind="ExternalOutput").ap()

    # mm2 target list per stage: (psum_name, w2_variant) ; variants:
    # 0 -> W2/2, 1 -> W2, 2 -> W2/6, 3 -> W2/3
    STAGE_TARGETS = [
        [("A", 0), ("S", 2)],  # K1: A1=(1/2)K1, S += (1/6)K1
        [("A", 0), ("S", 3)],  # K2
        [("A", 1), ("S", 3)],  # K3: A3=K3
        [("S", 2)],            # K4: S += (1/6)K4
    ]
    # cbv column per intermediate stage: c_i*b2' with c = [1/2, 1/2, 1]
    A_BIAS = [0, 0, 1]

    with tile.TileContext(nc) as tc, ExitStack() as es:
        consts = es.enter_context(tc.tile_pool(name="consts", bufs=1))
        w1s = consts.tile([128, 256], F16)
        w2s = consts.tile([128, 4, 256], F16)
        b1v = consts.tile([128, 2], F32)
        dtv = consts.tile([128, 1], F32)
        cbv = consts.tile([128, 2], F32)
        nc.sync.dma_start(w1s[:], W1S[:])
        nc.sync.dma_start(w2s[:], W2S[:])
        nc.sync.dma_start(b1v[:], B1V[:])
        nc.sync.dma_start(dtv[:], DTV[:])
        nc.sync.dma_start(cbv[:], CBV[:])

        xin_pool = es.enter_context(tc.tile_pool(name="xin", bufs=2))
        yst_pool = es.enter_context(tc.tile_pool(name="yst", bufs=7))
        yf_pool = es.enter_context(tc.tile_pool(name="yf", bufs=10))
        h_pool = es.enter_context(tc.tile_pool(name="h", bufs=10))
        zps_pool = es.enter_context(tc.tile_pool(name="zps", bufs=2, space="PSUM"))
        aps_pool = es.enter_context(tc.tile_pool(name="aps", bufs=1, space="PSUM"))
        sps_pool = es.enter_context(tc.tile_pool(name="sps", bufs=3, space="PSUM"))

        def mm1_wave(zw, yf, w):
            """z[hidden wave w] = W1_w @ y for both batch halves; concurrent
            rowgroup pair, fp32 PSUM [128, 1024] (2 banks)."""
            c = 128 * w
            for r in (0, 64):
                nc.tensor.matmul(
                    zw[:, 512 * (r // 64) : 512 * (r // 64) + 512],
                    w1s[r : r + 64, c : c + 128],
                    yf[r : r + 64, :],
                    start=True,
                    stop=True,
                    tile_position=(r, 0),
                    skip_group_check=True,
                )

        def mm2_wave(tgt, v, h, w, start, stop):
            """tgt[:, :] += s_v * W2_w @ h~_w  (col-tiled over batch halves,
            both reading the same h tile so the pair issues back-to-back)."""
            c = 128 * w
            for d in (0, 64):
                nc.tensor.matmul(
                    tgt[d : d + 64, :],
                    w2s[:, v, c + d : c + d + 64],
                    h[:, 512 * (d // 64) : 512 * (d // 64) + 512],
                    start=start,
                    stop=stop,
                    tile_position=(0, d),
                    skip_group_check=True,
                )

        def stage_group(sts, i, deferred):
            """One RK4 stage for the in-flight chunks at wave granularity.
            The S-target matmuls of stage i-1 (in `deferred`) are emitted
            after this stage's mm1s so the PE prioritizes z production;
            they only matter at the end of the step."""
            for st in sts:
                st["zw"] = [None, None]
                st["h"] = [None, None]
                if i < 3:
                    aps_t = aps_pool.tile([128, NT], F32, tag="aps")
                    st["aps"] = aps_t
            for w in (0, 1):
                for st in sts:
                    zw = zps_pool.tile([128, 2 * NT], F32, tag="zps")
                    st["zw"][w] = zw
                    mm1_wave(zw, st["rhs"], w)
                for st in sts:
                    # h~ = elup1(z + b1) in one ACT pass (patched exp table)
                    h = h_pool.tile([128, 2 * NT], F16, tag="h")
                    st["h"][w] = h
                    nc.scalar.activation(
                        h[:],
                        st["zw"][w][:],
                        mybir.ActivationFunctionType.Exp,
                        bias=b1v[:, w : w + 1],
                        scale=1.0,
                    )
            for emit in deferred:
                emit()
            deferred.clear()
            sv = [v for tname, v in STAGE_TARGETS[i] if tname == "S"][0]
            for st in sts:
                hs = st["h"]
                if i < 3:
                    av = [v for tname, v in STAGE_TARGETS[i] if tname == "A"][0]
                    for w in (0, 1):
                        mm2_wave(st["aps"], av, hs[w], w, start=w == 0, stop=w == 1)
                def emit_s(st=st, hs=hs, i=i, sv=sv):
                    for w in (0, 1):
                        mm2_wave(
                            st["sps"], sv, hs[w], w,
                            start=(i == 0 and w == 0),
                            stop=(i == 3 and w == 1),
                        )
                if i == 3:
                    emit_s()
                else:
                    deferred.append(emit_s)
            if i < 3:
                for st in sts:
                    # y_i = (A + c_i b2')*dt + y   (fp16, feeds next mm1)
                    ynext = yf_pool.tile([128, NT], F16, tag="yf")
                    nc.vector._custom_dve(
                        finup,
                        out=ynext,
                        in0=st["aps"][:],
                        in1=st["yf"],
                        s0=cbv[:, A_BIAS[i] : A_BIAS[i] + 1],
                        s1=dtv[:, 0:1],
                    )
                    st["rhs"] = ynext
            else:
                for st in sts:
                    # next step's fp16 base first (critical path into mm1) ...
                    ynf = yf_pool.tile([128, NT], F16, tag="yf")
                    nc.vector._custom_dve(
                        finup,
                        out=ynf,
                        in0=st["sps"][:],
                        in1=st["yst"],
                        s0=cbv[:, 1:2],
                        s1=dtv[:, 0:1],
                    )
                    st["next_yf"] = ynf
                for st in sts:
                    # ... then the fp32 master state off the critical path
                    ynew = yst_pool.tile([128, NT], F32, tag="yst")
                    nc.vector._custom_dve(
                        finup,
                        out=ynew,
                        in0=st["sps"][:],
                        in1=st["yst"],
                        s0=cbv[:, 1:2],
                        s1=dtv[:, 0:1],
                    )
                    st["yst"] = ynew

        def group_body(col0, n_in_group):
            xin = xin_pool.tile([128, GROUP * NT], F32, tag="xin")
            nc.sync.dma_start(
                xin[:, 0 : n_in_group * NT], X[:, bass.ds(col0, n_in_group * NT)]
            )
            sts = []
            for j in range(n_in_group):
                yst = xin[:, j * NT : (j + 1) * NT]
                yf = yf_pool.tile([128, NT], F16, tag="yf")
                nc.vector.tensor_copy(yf, yst)
                sts.append({"yst": yst, "yf": yf, "rhs": yf, "sps": None})
            deferred = []
            for s in range(n_steps):
                for st in sts:
                    sps_t = sps_pool.tile([128, NT], F32, tag="sps")
                    st["sps"] = sps_t
                for i in range(4):
                    stage_group(sts, i, deferred)
                if s < n_steps - 1:
                    for st in sts:
                        st["yf"] = st["next_yf"]
                        st["rhs"] = st["next_yf"]
            for j in range(n_in_group):
                nc.sync.dma_start(OUT[:, bass.ds(col0 + j * NT, NT)], sts[j]["yst"])

        if use_loop:
            with tc.For_i(
                0,
                N_GROUPS * GROUP * NT,
                GROUP * NT,
                hint_engines=(
                    mybir.EngineType.PE,
                    mybir.EngineType.Activation,
                    mybir.EngineType.DVE,
                ),
            ) as col0:
                group_body(col0, GROUP)
        else:
            for g in range(N_GROUPS):
                group_body(g * GROUP * NT, GROUP)
        tail = N_CHUNKS - N_GROUPS * GROUP
        if tail:
            group_body(N_GROUPS * GROUP * NT, tail)

    nc.compile()
    return nc


# ---------------------------------------------------------------------------
# Host side: prep, shard, run, gather
# ---------------------------------------------------------------------------


def _pack_state(xs):
    """[R, 64] fp32 (R batch rows) -> [128, R/2] feature-major pair-stacked."""
    r = xs.shape[0]
    t = xs.reshape(r // CHUNK, 2, NT, DIM)  # [chunks, pair, NT, 64]
    t = t.transpose(1, 3, 0, 2)             # [pair, 64, chunks, NT]
    return np.ascontiguousarray(t.reshape(2 * DIM, r // 2), dtype=np.float32)


def _unpack_state(ys, r):
    t = ys.reshape(2, DIM, r // CHUNK, NT).transpose(2, 0, 3, 1)
    return np.ascontiguousarray(t.reshape(r, DIM))


def _host_consts(t, W1, b1, W2, b2):
    dt = np.float32(np.asarray(t).reshape(-1)[0] / N_STEPS)
    W1T = W1.astype(np.float32).T  # [64, 256]
    W2T = W2.astype(np.float32).T  # [256, 64]

    w1s = np.zeros((128, 256), np.float32)
    w1s[0:64] = W1T
    w1s[64:128] = W1T

    scales = [0.5, 1.0, 1.0 / 6.0, 1.0 / 3.0]
    w2s = np.zeros((128, 4, 256), np.float32)
    for v, sc in enumerate(scales):
        for w in (0, 1):
            blk = sc * W2T[128 * w : 128 * (w + 1), :]  # [128, 64]
            w2s[:, v, 128 * w : 128 * w + 64] = blk
            w2s[:, v, 128 * w + 64 : 128 * w + 128] = blk

    b2p = b2.astype(np.float32) - W2.astype(np.float32).sum(axis=1)
    b2ps = np.concatenate([b2p, b2p])  # [128] pair-stacked

    b1v = b1.astype(np.float32).reshape(2, 128).T.copy()  # [:,w] = b1[128w:128w+128]
    dtv = np.full((128, 1), dt, np.float32)
    cbv = np.stack([0.5 * b2ps, b2ps], axis=1).astype(np.float32)

    f16 = lambda a: a.astype(np.float16)
    return {
        "w1s": f16(w1s),
        "w2s": f16(w2s),
        "b1v": np.ascontiguousarray(b1v, np.float32),
        "dtv": dtv,
        "cbv": np.ascontiguousarray(cbv, np.float32),
    }


_NC_CACHE = {}


def _get_program():
    key = (N_GROUPS, GROUP, N_STEPS)
    if key not in _NC_CACHE:
        _NC_CACHE[key] = build_ode_program()
    return _NC_CACHE[key]


def kernel(x, t, W1, b1, W2, b2, _trace=False, _trace_kwargs=None):
    assert x.shape == (BATCH, DIM)
    nc = _get_program()
    consts = _host_consts(t, W1, b1, W2, b2)
    in_maps = []
    for c in range(N_CORES):
        shard = x[c * SHARD : (c + 1) * SHARD]
        m = {"x": _pack_state(np.asarray(shard, np.float32))}
        m.update(consts)
        in_maps.append(m)
    kw = {}
    if _trace:
        kw = {"trace": True, "trace_kwargs": _trace_kwargs or {}}
    # The patched table must be visible to the neuronx-cc invocation that the
    # first execution triggers; restore the env afterwards so no other jax
    # compile in this process picks it up.
    prev = os.environ.get("BASS_ACT_ROOT_JSON_PATH")
    os.environ["BASS_ACT_ROOT_JSON_PATH"] = forge_act_root()
    try:
        res = run_bass_kernel_spmd(nc, in_maps, core_ids=list(range(N_CORES)), **kw)
    finally:
        if prev is None:
            os.environ.pop("BASS_ACT_ROOT_JSON_PATH", None)
        else:
            os.environ["BASS_ACT_ROOT_JSON_PATH"] = prev
    outs = [_unpack_state(res.results[c]["y"], SHARD) for c in range(N_CORES)]
    full = np.concatenate(outs, axis=0)
    if _trace:
        return full, res
    return full


if __name__ == "__main__":
    rng = np.random.default_rng(0)
    x = rng.normal(size=(BATCH, DIM)).astype(np.float32)
    t = np.array([0.5], np.float32)
    s1, s2 = 1 / np.sqrt(DIM), 1 / np.sqrt(HID)
    W1 = rng.uniform(-s1, s1, (HID, DIM)).astype(np.float32)
    b1 = rng.uniform(-s1, s1, (HID,)).astype(np.float32)
    W2 = rng.uniform(-s2, s2, (DIM, HID)).astype(np.float32)
    b2 = rng.uniform(-s2, s2, (DIM,)).astype(np.float32)
    y = kernel(x=x, t=t, W1=W1, b1=b1, W2=W2, b2=b2)
    print("out", y.shape, y.dtype, np.abs(y).mean())
